# revision 11
# baseline (speedup 1.0000x reference)
"""Trainium2 Bass kernel for nn_AttributeOperator (MoE-style routing).

Computes out[b] = relu(attr_ops[attrs[b]] @ obj_emb[objs[b]]) for b in [0, B).

Strategy (expert-parallel): the dominant cost is streaming the attr_ops table
(N_ATTRS x D x D fp32 = 512 MB). Samples are grouped by attribute on the host,
groups are load-balanced across the 8 cores, and each core streams only its own
subset of operator matrices from HBM exactly once. Per group the core computes
X @ A^T via TensorE (X^T stationary, A^T streaming at N=512) accumulating over
the 4 K-chunks of 128, applies ReLU on ScalarE, and DMAs the rows out. The host
scatters rows back to their original batch positions.

attr_ops matrices are pre-transposed on the host so the contraction dim (j) is
the SBUF partition dim, making the device DMA fully contiguous.
"""

import numpy as np

import concourse.bass as bass
import concourse.tile as tile
from concourse import bacc, mybir
from concourse.bass_utils import run_bass_kernel_spmd

N_CORES = 8
D = 512               # embedding dim (hardcoded per problem spec)
QCH = D // 128        # contraction chunks of 128 partitions

# test.py hooks (ignored by the grading harness)
LAST_RESULTS = None   # BassKernelResults of the most recent run
TRACE = False
TRACE_CORES = None

_NC_CACHE = {}


def _build_nc(nm, cmax, ops_bufs=8, pair=1, sync_frac=(5, 8), reps=1):
    """Build + compile the SPMD program: nm matrix slots, cmax samples/slot.

    pair: matrices loaded per ops DMA (amortizes per-DMA fixed costs).
    sync_frac: (a, b) -> a of every b ops DMAs issue on sync, rest on scalar.
    reps: hardware-loop repetitions of the whole kernel (for timing).
    """
    ncol = nm * cmax  # columns of X^T per K-chunk
    nmp = -(-nm // pair) * pair  # nm rounded up to a multiple of pair
    nc = bacc.Bacc("TRN2", target_bir_lowering=False, debug=False,
                   num_devices=N_CORES)
    ops_dram = nc.dram_tensor("ops_t", [nmp, QCH, 128, D], mybir.dt.float32r,
                              kind="ExternalInput").ap()
    xt_dram = nc.dram_tensor("xt", [128, QCH * ncol], mybir.dt.float32r,
                             kind="ExternalInput").ap()
    out_dram = nc.dram_tensor("out", [ncol, D], mybir.dt.float32,
                              kind="ExternalOutput").ap()

    with tile.TileContext(nc) as tc:
        with (
            tc.tile_pool(name="xt", bufs=1) as xt_pool,
            tc.tile_pool(name="ops", bufs=ops_bufs) as ops_pool,
            tc.tile_pool(name="ps", bufs=8, space="PSUM") as ps_pool,
            tc.tile_pool(name="o", bufs=4) as o_pool,
        ):
            def body():
                xt_sb = xt_pool.tile([128, QCH * ncol], mybir.dt.float32r)
                nc.sync.dma_start(xt_sb[:], xt_dram[:])

                for g in range(nmp // pair):
                    m = ops_pool.tile([128, pair * QCH * D],
                                      mybir.dt.float32r, tag="m")
                    issuer = nc.sync if g % sync_frac[1] < sync_frac[0] \
                        else nc.scalar
                    issuer.dma_start(
                        m[:].rearrange("p (t q i) -> p t q i", t=pair, q=QCH),
                        ops_dram[g * pair:(g + 1) * pair].rearrange(
                            "t q p i -> p t q i"))
                    for t in range(pair):
                        s = g * pair + t
                        if s >= nm:
                            break
                        ps = ps_pool.tile([cmax, D], mybir.dt.float32,
                                          tag="ps")
                        for q in range(QCH):
                            lhsT = xt_sb[:, q * ncol + s * cmax:
                                         q * ncol + (s + 1) * cmax]
                            rhs = m[:, (t * QCH + q) * D:
                                    (t * QCH + q + 1) * D]
                            nc.tensor.matmul(ps[:], lhsT, rhs,
                                             start=(q == 0),
                                             stop=(q == QCH - 1))
                        o = o_pool.tile([cmax, D], mybir.dt.float32, tag="o")
                        nc.vector.tensor_scalar_max(o[:], ps[:], 0.0)
                        nc.scalar.dma_start(
                            out_dram[s * cmax:(s + 1) * cmax, :], o[:])

            if reps == 1:
                body()
            else:
                with tc.For_i(0, reps, 1,
                              hint_engines=(mybir.EngineType.PE,)):
                    body()

    nc.compile()
    return nc


def _route(attrs):
    """Group sample indices by attribute, chunk to <=128, snake-balance
    across cores. Returns per-core slot lists of (attr_id, idx_array)."""
    order = np.argsort(attrs, kind="stable")
    sorted_attrs = attrs[order]
    uniq, starts, counts = np.unique(sorted_attrs, return_index=True,
                                     return_counts=True)
    chunks = []
    for a, st, c in zip(uniq, starts, counts):
        idx = order[st:st + c]
        for o in range(0, c, 128):
            chunks.append((int(a), idx[o:o + 128]))
    chunks.sort(key=lambda t: -len(t[1]))
    per_core = [[] for _ in range(N_CORES)]
    for i, ch in enumerate(chunks):
        r, pos = divmod(i, N_CORES)
        k = pos if r % 2 == 0 else N_CORES - 1 - pos
        per_core[k].append(ch)
    return per_core


PAIR = 1


def _prepare(attrs, objs, attr_ops, obj_emb):
    """Route + build per-core device input maps."""
    per_core = _route(attrs)
    nm = max(1, max(len(s) for s in per_core))
    cmax = max(1, max((len(ix) for s in per_core for _, ix in s), default=1))
    ncol = nm * cmax
    nmp = -(-nm // PAIR) * PAIR

    rep = obj_emb[objs]  # [B, D] object representations
    in_maps = []
    for k in range(N_CORES):
        slots = per_core[k]
        ops_t = np.zeros((nmp, D, D), np.float32)
        r = np.zeros((ncol, D), np.float32)
        for s, (a, idx) in enumerate(slots):
            np.copyto(ops_t[s], attr_ops[a].T)
            r[s * cmax:s * cmax + len(idx)] = rep[idx]
        # xt[p, q*ncol + c] = r[c, q*128 + p]
        xt = np.ascontiguousarray(
            r.reshape(ncol, QCH, 128).transpose(2, 1, 0)).reshape(128, -1)
        in_maps.append({"ops_t": ops_t.reshape(nmp, QCH, 128, D), "xt": xt})
    return per_core, nm, cmax, in_maps


def kernel(attrs, objs, attr_ops, obj_emb):
    global LAST_RESULTS
    attrs = np.asarray(attrs)
    objs = np.asarray(objs)
    attr_ops = np.asarray(attr_ops, dtype=np.float32)
    obj_emb = np.asarray(obj_emb, dtype=np.float32)
    B = attrs.shape[0]
    d = obj_emb.shape[1]
    assert d == D and attr_ops.shape[1:] == (D, D)

    per_core, nm, cmax, in_maps = _prepare(attrs, objs, attr_ops, obj_emb)

    nc = _NC_CACHE.get((nm, cmax))
    if nc is None:
        nc = _NC_CACHE[(nm, cmax)] = _build_nc(nm, cmax, pair=PAIR)

    res = run_bass_kernel_spmd(nc, in_maps, core_ids=list(range(N_CORES)),
                               trace=TRACE, trace_cores=TRACE_CORES)
    LAST_RESULTS = res

    out = np.zeros((B, d), np.float32)
    for k in range(N_CORES):
        out_k = res.results[k]["out"]
        for s, (a, idx) in enumerate(per_core[k]):
            out[idx] = out_k[s * cmax:s * cmax + len(idx)]
    return out


# revision 16
# speedup vs baseline: 1.1539x; 1.1539x over previous
"""Trainium2 Bass kernel for nn_AttributeOperator (MoE-style routing).

Computes out[b] = relu(attr_ops[attrs[b]] @ obj_emb[objs[b]]) for b in [0, B).

Strategy (expert-parallel): the dominant cost is streaming the attr_ops table
(N_ATTRS x D x D fp32 = 512 MB). Samples are grouped by attribute on the host,
groups are load-balanced across the 8 cores (snake deal by group size), and
each core streams only its own subset of operator matrices from HBM exactly
once. Per group the core computes X @ A^T on TensorE (X^T stationary, A^T
streaming at N=512, fp32r) accumulating over the 4 K-chunks of 128, applies
ReLU on VectorE, and DMAs the rows out. The host scatters rows back to their
original batch positions.

attr_ops matrices are pre-transposed on the host so the contraction dim (j) is
the SBUF partition dim, making the device DMA fully contiguous. The SPMD
program is identical on all 8 cores; only the per-core input tensors differ.
Slot s has a fixed column capacity maxc[s] = max over cores of that rank's
group size, so the one program fits every core's routing.
"""

import numpy as np

import concourse.bass as bass
import concourse.tile as tile
from concourse import bacc, mybir
from concourse.bass_utils import run_bass_kernel_spmd

N_CORES = 8
D = 512               # embedding dim (hardcoded per problem spec)
QCH = D // 128        # contraction chunks of 128 partitions

# test.py hooks (ignored by the grading harness)
LAST_RESULTS = None   # BassKernelResults of the most recent run
TRACE = False
TRACE_CORES = None

PAIR = 1
_NC_CACHE = {}


def _build_nc(maxc, offs, ncol, ops_bufs=8, pair=1, sync_frac=(0, 1), reps=1,
              out_engine="sync", staggered=False):
    """Build + compile the SPMD program.

    maxc[s]: column capacity of slot s; offs[s]: column offset of slot s;
    ncol: total columns (= offs[-1] + maxc[-1]).
    pair: matrices loaded per ops DMA (amortizes per-DMA fixed costs).
    sync_frac: (a, b) -> a of every b ops DMAs issue on sync, rest on scalar.
    reps: hardware-loop repetitions of the whole kernel (for timing).
    """
    nm = len(maxc)
    nmp = -(-nm // pair) * pair  # nm rounded up to a multiple of pair
    nc = bacc.Bacc("TRN2", target_bir_lowering=False, debug=False,
                   num_devices=N_CORES)
    ops_dram = nc.dram_tensor("ops_t", [nmp, QCH, 128, D], mybir.dt.float32r,
                              kind="ExternalInput").ap()
    xt_dram = nc.dram_tensor("xt", [128, QCH * ncol], mybir.dt.float32r,
                             kind="ExternalInput").ap()
    out_dram = nc.dram_tensor("out", [ncol, D], mybir.dt.float32,
                              kind="ExternalOutput").ap()

    with tile.TileContext(nc) as tc:
        with (
            tc.tile_pool(name="xt", bufs=1) as xt_pool,
            tc.tile_pool(name="ops", bufs=ops_bufs) as ops_pool,
            tc.tile_pool(name="ps", bufs=8, space="PSUM") as ps_pool,
            tc.tile_pool(name="o", bufs=4) as o_pool,
        ):
            def body():
                xt_sb = xt_pool.tile([128, QCH * ncol], mybir.dt.float32r)
                nc.sync.dma_start(xt_sb[:], xt_dram[:])

                for g in range(nmp // pair):
                    m = ops_pool.tile([128, pair * QCH * D],
                                      mybir.dt.float32r, tag="m")
                    issuer = nc.sync if g % sync_frac[1] < sync_frac[0] \
                        else nc.scalar
                    issuer.dma_start(
                        m[:].rearrange("p (t q i) -> p t q i", t=pair, q=QCH),
                        ops_dram[g * pair:(g + 1) * pair].rearrange(
                            "t q p i -> p t q i"))
                    for t in range(pair):
                        s = g * pair + t
                        if s >= nm:
                            break
                        cw = maxc[s]
                        ps = ps_pool.tile([cw, D], mybir.dt.float32, tag="ps")
                        for q in range(QCH):
                            lhsT = xt_sb[:, q * ncol + offs[s]:
                                         q * ncol + offs[s] + cw]
                            rhs = m[:, (t * QCH + q) * D:
                                    (t * QCH + q + 1) * D]
                            nc.tensor.matmul(ps[:], lhsT, rhs,
                                             start=(q == 0),
                                             stop=(q == QCH - 1))
                        o = o_pool.tile([cw, D], mybir.dt.float32, tag="o")
                        nc.vector.tensor_scalar_max(o[:], ps[:], 0.0)
                        out_eng = getattr(nc, out_engine)
                        out_eng.dma_start(
                            out_dram[offs[s]:offs[s] + cw, :], o[:])

            if reps == 1:
                body()
            else:
                with tc.For_i(0, reps, 1,
                              hint_engines=(mybir.EngineType.PE,),
                              staggered_reset=staggered):
                    body()

    nc.compile()
    return nc


def _route(attrs):
    """Group sample indices by attribute, chunk to <=128, snake-balance
    across cores. Returns per-core slot lists of (attr_id, idx_array),
    each list sorted by descending group size."""
    order = np.argsort(attrs, kind="stable")
    sorted_attrs = attrs[order]
    uniq, starts, counts = np.unique(sorted_attrs, return_index=True,
                                     return_counts=True)
    chunks = []
    for a, st, c in zip(uniq, starts, counts):
        idx = order[st:st + c]
        for o in range(0, c, 128):
            chunks.append((int(a), idx[o:o + 128]))
    chunks.sort(key=lambda t: -len(t[1]))
    per_core = [[] for _ in range(N_CORES)]
    for i, ch in enumerate(chunks):
        r, pos = divmod(i, N_CORES)
        k = pos if r % 2 == 0 else N_CORES - 1 - pos
        per_core[k].append(ch)
    return per_core


def _layout(per_core):
    """Per-slot-rank column capacity/offset shared by all cores."""
    nm = max(1, max(len(s) for s in per_core))
    maxc = [1] * nm
    for slots in per_core:
        for s, (_, idx) in enumerate(slots):
            maxc[s] = max(maxc[s], len(idx))
    offs = [0] * nm
    for s in range(1, nm):
        offs[s] = offs[s - 1] + maxc[s - 1]
    ncol = offs[-1] + maxc[-1]
    return nm, maxc, offs, ncol


def _prepare(attrs, objs, attr_ops, obj_emb):
    """Route + build per-core device input maps."""
    per_core = _route(attrs)
    nm, maxc, offs, ncol = _layout(per_core)
    nmp = -(-nm // PAIR) * PAIR

    rep = obj_emb[objs]  # [B, D] object representations
    in_maps = []
    for k in range(N_CORES):
        slots = per_core[k]
        ops_t = np.zeros((nmp, D, D), np.float32)
        r = np.zeros((ncol, D), np.float32)
        for s, (a, idx) in enumerate(slots):
            np.copyto(ops_t[s], attr_ops[a].T)
            r[offs[s]:offs[s] + len(idx)] = rep[idx]
        # xt[p, q*ncol + c] = r[c, q*128 + p]
        xt = np.ascontiguousarray(
            r.reshape(ncol, QCH, 128).transpose(2, 1, 0)).reshape(128, -1)
        in_maps.append({"ops_t": ops_t.reshape(nmp, QCH, 128, D), "xt": xt})
    return per_core, (nm, tuple(maxc), tuple(offs), ncol), in_maps


def kernel(attrs, objs, attr_ops, obj_emb):
    global LAST_RESULTS
    attrs = np.asarray(attrs)
    objs = np.asarray(objs)
    attr_ops = np.asarray(attr_ops, dtype=np.float32)
    obj_emb = np.asarray(obj_emb, dtype=np.float32)
    B = attrs.shape[0]
    d = obj_emb.shape[1]
    assert d == D and attr_ops.shape[1:] == (D, D)

    per_core, (nm, maxc, offs, ncol), in_maps = _prepare(
        attrs, objs, attr_ops, obj_emb)

    nc = _NC_CACHE.get(maxc)
    if nc is None:
        nc = _NC_CACHE[maxc] = _build_nc(maxc, offs, ncol, pair=PAIR)

    res = run_bass_kernel_spmd(nc, in_maps, core_ids=list(range(N_CORES)),
                               trace=TRACE, trace_cores=TRACE_CORES)
    LAST_RESULTS = res

    out = np.zeros((B, d), np.float32)
    for k in range(N_CORES):
        out_k = res.results[k]["out"]
        for s, (a, idx) in enumerate(per_core[k]):
            out[idx] = out_k[offs[s]:offs[s] + len(idx)]
    return out


# revision 18
# speedup vs baseline: 1.2219x; 1.0590x over previous
"""Trainium2 Bass kernel for nn_AttributeOperator (MoE-style routing).

Computes out[b] = relu(attr_ops[attrs[b]] @ obj_emb[objs[b]]) for b in [0, B).

Strategy (expert-parallel): the dominant cost is streaming the attr_ops table
(N_ATTRS x D x D fp32 = 512 MB). Samples are grouped by attribute on the host,
groups are load-balanced across the 8 cores (snake deal by group size), and
each core streams only its own subset of operator matrices from HBM exactly
once. Per group the core computes X @ A^T on TensorE (X^T stationary, A^T
streaming at N=512, fp32r) accumulating over the 4 K-chunks of 128, applies
ReLU on VectorE, and DMAs the rows out. The host scatters rows back to their
original batch positions.

attr_ops matrices are pre-transposed on the host so the contraction dim (j) is
the SBUF partition dim, making the device DMA fully contiguous. The SPMD
program is identical on all 8 cores; only the per-core input tensors differ.
Slot s has a fixed column capacity maxc[s] = max over cores of that rank's
group size, so the one program fits every core's routing.
"""

import numpy as np

import concourse.tile as tile
from concourse import bacc, mybir
from concourse.bass_utils import run_bass_kernel_spmd

N_CORES = 8
D = 512               # embedding dim (hardcoded per problem spec)
QCH = D // 128        # contraction chunks of 128 partitions

# test.py hooks (ignored by the grading harness)
LAST_RESULTS = None   # BassKernelResults of the most recent run
TRACE = False
TRACE_CORES = None

PAIR = 1
_NC_CACHE = {}


def _build_nc(maxc, offs, ncol, ops_bufs=8, pair=1, sync_frac=(0, 1), reps=1,
              out_engine="sync", staggered=False):
    """Build + compile the SPMD program.

    maxc[s]: column capacity of slot s; offs[s]: column offset of slot s;
    ncol: total columns (= offs[-1] + maxc[-1]).
    pair: matrices loaded per ops DMA (amortizes per-DMA fixed costs).
    sync_frac: (a, b) -> a of every b ops DMAs issue on sync, rest on scalar.
    reps: hardware-loop repetitions of the whole kernel (for timing).
    staggered: staggered-reset loop back-edge — wedges this device, keep False.
    """
    nm = len(maxc)
    nmp = -(-nm // pair) * pair  # nm rounded up to a multiple of pair
    nc = bacc.Bacc("TRN2", target_bir_lowering=False, debug=False,
                   num_devices=N_CORES)
    ops_dram = nc.dram_tensor("ops_t", [nmp, QCH, 128, D], mybir.dt.float32r,
                              kind="ExternalInput").ap()
    xt_dram = nc.dram_tensor("xt", [128, QCH * ncol], mybir.dt.float32r,
                             kind="ExternalInput").ap()
    out_dram = nc.dram_tensor("out", [ncol, D], mybir.dt.float32,
                              kind="ExternalOutput").ap()

    with tile.TileContext(nc) as tc:
        with (
            tc.tile_pool(name="xt", bufs=1) as xt_pool,
            tc.tile_pool(name="ops", bufs=ops_bufs) as ops_pool,
            tc.tile_pool(name="ps", bufs=8, space="PSUM") as ps_pool,
            tc.tile_pool(name="o", bufs=4) as o_pool,
        ):
            def body():
                xt_sb = xt_pool.tile([128, QCH * ncol], mybir.dt.float32r)
                nc.sync.dma_start(xt_sb[:], xt_dram[:])

                for g in range(nmp // pair):
                    m = ops_pool.tile([128, pair * QCH * D],
                                      mybir.dt.float32r, tag="m")
                    issuer = nc.sync if g % sync_frac[1] < sync_frac[0] \
                        else nc.scalar
                    issuer.dma_start(
                        m[:].rearrange("p (t q i) -> p t q i", t=pair, q=QCH),
                        ops_dram[g * pair:(g + 1) * pair].rearrange(
                            "t q p i -> p t q i"))
                    for t in range(pair):
                        s = g * pair + t
                        if s >= nm:
                            break
                        cw = maxc[s]
                        ps = ps_pool.tile([cw, D], mybir.dt.float32, tag="ps")
                        for q in range(QCH):
                            lhsT = xt_sb[:, q * ncol + offs[s]:
                                         q * ncol + offs[s] + cw]
                            rhs = m[:, (t * QCH + q) * D:
                                    (t * QCH + q + 1) * D]
                            nc.tensor.matmul(ps[:], lhsT, rhs,
                                             start=(q == 0),
                                             stop=(q == QCH - 1))
                        o = o_pool.tile([cw, D], mybir.dt.float32, tag="o")
                        nc.vector.tensor_scalar_max(o[:], ps[:], 0.0)
                        out_eng = getattr(nc, out_engine)
                        out_eng.dma_start(
                            out_dram[offs[s]:offs[s] + cw, :], o[:])

            if reps == 1:
                body()
            else:
                with tc.For_i(0, reps, 1,
                              hint_engines=(mybir.EngineType.PE,),
                              staggered_reset=staggered):
                    body()

    nc.compile()
    return nc


def _route(attrs):
    """Group sample indices by attribute, chunk to <=128, snake-balance
    across cores. Returns per-core slot lists of (attr_id, idx_array),
    each list sorted by descending group size."""
    order = np.argsort(attrs, kind="stable")
    sorted_attrs = attrs[order]
    uniq, starts, counts = np.unique(sorted_attrs, return_index=True,
                                     return_counts=True)
    chunks = []
    for a, st, c in zip(uniq, starts, counts):
        idx = order[st:st + c]
        for o in range(0, c, 128):
            chunks.append((int(a), idx[o:o + 128]))
    chunks.sort(key=lambda t: -len(t[1]))
    per_core = [[] for _ in range(N_CORES)]
    for i, ch in enumerate(chunks):
        r, pos = divmod(i, N_CORES)
        k = pos if r % 2 == 0 else N_CORES - 1 - pos
        per_core[k].append(ch)
    return per_core


def _layout(per_core):
    """Per-slot-rank column capacity/offset shared by all cores."""
    nm = max(1, max(len(s) for s in per_core))
    maxc = [1] * nm
    for slots in per_core:
        for s, (_, idx) in enumerate(slots):
            maxc[s] = max(maxc[s], len(idx))
    offs = [0] * nm
    for s in range(1, nm):
        offs[s] = offs[s - 1] + maxc[s - 1]
    ncol = offs[-1] + maxc[-1]
    return nm, maxc, offs, ncol


def _prepare(attrs, objs, attr_ops, obj_emb):
    """Route + build per-core device input maps."""
    per_core = _route(attrs)
    nm, maxc, offs, ncol = _layout(per_core)
    nmp = -(-nm // PAIR) * PAIR

    rep = obj_emb[objs]  # [B, D] object representations
    in_maps = []
    for k in range(N_CORES):
        slots = per_core[k]
        ops_t = np.zeros((nmp, D, D), np.float32)
        r = np.zeros((ncol, D), np.float32)
        for s, (a, idx) in enumerate(slots):
            np.copyto(ops_t[s], attr_ops[a].T)
            r[offs[s]:offs[s] + len(idx)] = rep[idx]
        # xt[p, q*ncol + c] = r[c, q*128 + p]
        xt = np.ascontiguousarray(
            r.reshape(ncol, QCH, 128).transpose(2, 1, 0)).reshape(128, -1)
        in_maps.append({"ops_t": ops_t.reshape(nmp, QCH, 128, D), "xt": xt})
    return per_core, (nm, tuple(maxc), tuple(offs), ncol), in_maps


def kernel(attrs, objs, attr_ops, obj_emb):
    global LAST_RESULTS
    attrs = np.asarray(attrs)
    objs = np.asarray(objs)
    attr_ops = np.asarray(attr_ops, dtype=np.float32)
    obj_emb = np.asarray(obj_emb, dtype=np.float32)
    B = attrs.shape[0]
    d = obj_emb.shape[1]
    assert d == D and attr_ops.shape[1:] == (D, D)

    per_core, (nm, maxc, offs, ncol), in_maps = _prepare(
        attrs, objs, attr_ops, obj_emb)

    nc = _NC_CACHE.get(maxc)
    if nc is None:
        nc = _NC_CACHE[maxc] = _build_nc(maxc, offs, ncol, pair=PAIR)

    res = run_bass_kernel_spmd(nc, in_maps, core_ids=list(range(N_CORES)),
                               trace=TRACE, trace_cores=TRACE_CORES)
    LAST_RESULTS = res

    out = np.zeros((B, d), np.float32)
    for k in range(N_CORES):
        out_k = res.results[k]["out"]
        for s, (a, idx) in enumerate(per_core[k]):
            out[idx] = out_k[offs[s]:offs[s] + len(idx)]
    return out
